# revision 9
# baseline (speedup 1.0000x reference)
"""Trainium2 Bass kernel for GQA attention (nn_Attention_39676907888265).

Model: B=4, T=2048, D=2048, 16 heads / 4 query groups, head_dim=128,
rotate-half RoPE, non-causal attention (all-ones key padding mask),
fused QKV projection and output projection.

Sharding (8 NeuronCores): data-parallel over batch (4) x tensor-parallel
over heads (2).  Core c handles batch c%4 and head-half c//4 (8 query
heads, 2 KV groups).  Each core computes a partial o_proj output
(row-parallel over the head dimension); the host sums the two partials
per batch during unshard (the "all-reduce after o_proj" done at gather).

Per-core kernel (all-transposed layouts, fp32r matmuls):
  pass A: K^T,V projection for all T (RoPE on K fused into PSUM drain)
  pass B, per 512-query chunk: Q^T projection + RoPE, then per head
    S^T = K_h @ Q_h^T on PE -> exp via ScalarE (scale=1/sqrt(d)) -> P^T,
    PV and all-ones (row-sum, pre-broadcast) matmuls accumulate in PSUM,
    normalize via VectorE reciprocal+multiply, then o_proj partial.
Rotate-half is done with partition-offset SBUF->SBUF DMA copies (the
sign of sin is folded into the host-built sin table).
"""

import math

import numpy as np

# ---------------------------------------------------------------- constants
B, T, D = 4, 2048, 2048
NH, QG, HD = 16, 4, 128
HPG = NH // QG  # heads per group
THETA = 10000.0
SCALE = 0.08838834764831845
N_CORES = 8
TP = 2                      # head-parallel ways
LOCAL_H = NH // TP          # 8 query heads per core
LOCAL_G = QG // TP          # 2 kv groups per core
CHUNK = 512                 # query/time chunk
NCHUNK = T // CHUNK         # 4
KT = D // 128               # 16 contraction tiles for projections
TT = T // 128               # 16 key tiles


def _build_nc(iters: int = 1, split_waits: bool = True):
    import concourse.bass as bass
    import concourse.mybir as mybir
    import concourse.tile as tile

    f32 = mybir.dt.float32
    f32r = mybir.dt.float32r


    nc = bass.Bass("TRN2", target_bir_lowering=False, debug=False)

    xT = nc.dram_tensor("xT", [D, T], f32r, kind="ExternalInput")
    wq = nc.dram_tensor("wq", [D, LOCAL_H * HD], f32r, kind="ExternalInput")
    wk = nc.dram_tensor("wk", [D, LOCAL_G * HD], f32r, kind="ExternalInput")
    wv = nc.dram_tensor("wv", [D, LOCAL_G * HD], f32r, kind="ExternalInput")
    wo = nc.dram_tensor("wo", [LOCAL_H * HD, D], f32r, kind="ExternalInput")
    cosT = nc.dram_tensor("cosT", [HD, T], f32, kind="ExternalInput")
    sinT = nc.dram_tensor("sinT", [HD, T], f32, kind="ExternalInput")  # sign-folded
    yT = nc.dram_tensor("yT", [D, T], f32, kind="ExternalOutput")

    with tile.TileContext(nc) as tc:
        with (
            tc.tile_pool(name="persist", bufs=1) as persist,
            tc.tile_pool(name="xs", bufs=1) as xs_pool,
            tc.tile_pool(name="wstream", bufs=2) as wstream,
            tc.tile_pool(name="wostream", bufs=2) as wostream,
            tc.tile_pool(name="qchunk", bufs=1) as qchunk_pool,
            tc.tile_pool(name="ochunk", bufs=1) as ochunk_pool,
            tc.tile_pool(name="pt", bufs=2) as pt_pool,
            tc.tile_pool(name="small", bufs=2) as small,
            tc.tile_pool(name="ropetmp", bufs=2) as ropetmp,
            tc.tile_pool(name="ydrain", bufs=2) as ydrain,
            tc.tile_pool(name="psA", bufs=2, space="PSUM") as psA,
            tc.tile_pool(name="psS", bufs=2, space="PSUM") as psS,
            tc.tile_pool(name="psO", bufs=1, space="PSUM") as psO,
            tc.tile_pool(name="psR", bufs=1, space="PSUM") as psR,
        ):
            # ---------------- resident tensors
            cos_s = persist.tile([HD, T], f32)
            sin_s = persist.tile([HD, T], f32)
            nc.sync.dma_start(out=cos_s, in_=cosT[:, :])
            nc.sync.dma_start(out=sin_s, in_=sinT[:, :])
            ones_s = persist.tile([128, 128], f32r)
            ones_f = persist.tile([128, 128], f32)
            nc.vector.memset(ones_f, 1.0)
            nc.vector.tensor_copy(ones_s, ones_f)
            K_s = persist.tile([128, LOCAL_G, T], f32r)       # [d, kv-head, t]
            V_s = persist.tile([128, TT, LOCAL_G * HD], f32r)  # [t%128, t//128, e]
            wk_s = persist.tile([128, KT, LOCAL_G * HD], f32r)
            wv_s = persist.tile([128, KT, LOCAL_G * HD], f32r)
            nc.sync.dma_start(out=wk_s, in_=wk.rearrange("(kt p) e -> p kt e", p=128))
            nc.sync.dma_start(out=wv_s, in_=wv.rearrange("(kt p) e -> p kt e", p=128))

            def rope_drain(ps, out_ap, tmp_shape, c0, c1):
                """out = ps*cos + rot_half(ps)*sin_signed over columns [c0:c1)."""
                raw = ropetmp.tile(tmp_shape, f32, tag="raw")
                swp = ropetmp.tile(tmp_shape, f32, tag="swp")
                nc.vector.tensor_copy(raw, ps)
                h = HD // 2
                nc.sync.dma_start(out=swp[0:h, :], in_=raw[h:HD, :])
                nc.sync.dma_start(out=swp[h:HD, :], in_=raw[0:h, :])
                nc.vector.tensor_mul(raw, raw, cos_s[:, c0:c1])
                nc.vector.tensor_mul(swp, swp, sin_s[:, c0:c1])
                nc.vector.tensor_add(out_ap, raw, swp)

            def body(iv):
                # ---------------- pass A: K^T (+RoPE) and V for all T
                for c in range(NCHUNK):
                    c0, c1 = c * CHUNK, (c + 1) * CHUNK
                    xt = xs_pool.tile([128, KT, CHUNK], f32r, tag="x")
                    nc.sync.dma_start(
                        out=xt,
                        in_=xT[:, c0:c1].rearrange("(kt p) t -> p kt t", p=128),
                    )
                    for g in range(LOCAL_G):
                        ps = psA.tile([128, CHUNK], f32, tag="ps")
                        for kt in range(KT):
                            nc.tensor.matmul(
                                ps,
                                wk_s[:, kt, g * HD:(g + 1) * HD],
                                xt[:, kt, :],
                                start=(kt == 0), stop=(kt == KT - 1),
                            )
                        rope_drain(ps, K_s[:, g, c0:c1], [HD, CHUNK], c0, c1)
                    for tt in range(CHUNK // 128):
                        ps = psA.tile([128, LOCAL_G * HD], f32, tag="ps")
                        for kt in range(KT):
                            nc.tensor.matmul(
                                ps,
                                xt[:, kt, tt * 128:(tt + 1) * 128],
                                wv_s[:, kt, :],
                                start=(kt == 0), stop=(kt == KT - 1),
                            )
                        nc.vector.tensor_copy(V_s[:, c * (CHUNK // 128) + tt, :], ps)

                # ---------------- pass B: per chunk Q -> attention -> o_proj
                for c in range(NCHUNK):
                    c0, c1 = c * CHUNK, (c + 1) * CHUNK
                    xt = xs_pool.tile([128, KT, CHUNK], f32r, tag="x")
                    nc.sync.dma_start(
                        out=xt,
                        in_=xT[:, c0:c1].rearrange("(kt p) t -> p kt t", p=128),
                    )
                    Q_c = qchunk_pool.tile([128, LOCAL_H, CHUNK], f32r)
                    for h in range(LOCAL_H):
                        wq_t = wstream.tile([128, KT, HD], f32r, tag="wq")
                        nc.sync.dma_start(
                            out=wq_t,
                            in_=wq[:, h * HD:(h + 1) * HD].rearrange(
                                "(kt p) e -> p kt e", p=128),
                        )
                        ps = psA.tile([128, CHUNK], f32, tag="ps")
                        for kt in range(KT):
                            nc.tensor.matmul(
                                ps, wq_t[:, kt, :], xt[:, kt, :],
                                start=(kt == 0), stop=(kt == KT - 1),
                            )
                        rope_drain(ps, Q_c[:, h, :], [HD, CHUNK], c0, c1)

                    O_c = ochunk_pool.tile([128, LOCAL_H, CHUNK], f32r)
                    for h in range(LOCAL_H):
                        g = h // HPG  # local kv group of this local head
                        O_ps = psO.tile([128, CHUNK], f32)
                        R_ps = psR.tile([128, CHUNK], f32)
                        for b2 in range(TT // 2):  # batches of 2 key tiles
                            ST = psS.tile([128, 2, CHUNK], f32)
                            for j in range(2):
                                kt = b2 * 2 + j
                                nc.tensor.matmul(
                                    ST[:, j, :],
                                    K_s[:, g, kt * 128:(kt + 1) * 128],
                                    Q_c[:, h, :],
                                    start=True, stop=True,
                                )
                            PT = pt_pool.tile([128, 2, CHUNK], f32r)
                            nc.scalar.activation(
                                PT, ST,
                                func=mybir.ActivationFunctionType.Exp,
                                scale=SCALE,
                            )
                            for j in range(2):
                                kt = b2 * 2 + j
                                nc.tensor.matmul(
                                    O_ps,
                                    V_s[:, kt, g * HD:(g + 1) * HD],
                                    PT[:, j, :],
                                    start=(kt == 0), stop=(kt == TT - 1),
                                )
                                nc.tensor.matmul(
                                    R_ps,
                                    ones_s,
                                    PT[:, j, :],
                                    start=(kt == 0), stop=(kt == TT - 1),
                                )
                        recip = small.tile([128, CHUNK], f32, tag="recip")
                        nc.vector.reciprocal(recip, R_ps)
                        nc.vector.tensor_mul(O_c[:, h, :], O_ps, recip)

                    for et in range(D // 128):
                        wo_t = wostream.tile([128, LOCAL_H, 128], f32r, tag="wo")
                        nc.sync.dma_start(
                            out=wo_t,
                            in_=wo[:, et * 128:(et + 1) * 128].rearrange(
                                "(dt p) e -> p dt e", p=128),
                        )
                        y_ps = psA.tile([128, CHUNK], f32, tag="ps")
                        for h in range(LOCAL_H):
                            nc.tensor.matmul(
                                y_ps, wo_t[:, h, :], O_c[:, h, :],
                                start=(h == 0), stop=(h == LOCAL_H - 1),
                            )
                        y_s = ydrain.tile([128, CHUNK], f32, tag="y")
                        nc.vector.tensor_copy(y_s, y_ps)
                        nc.sync.dma_start(
                            out=yT[et * 128:(et + 1) * 128, c0:c1], in_=y_s)

            if iters > 1:
                with tc.For_i(0, iters, 1) as iv:
                    body(iv)
            else:
                body(0)

    if split_waits:
        _split_multi_waits(nc)
    return nc


def _split_multi_waits(nc):
    """This container's walrus rejects >1 sync wait per instruction; split
    extras into single-wait NOPs on the same engine just before it."""
    import bass_rust

    count = 0
    for f in nc.m.functions:
        for blk in f.blocks:
            new = []
            for inst in blk.instructions:
                si = inst.sync_info
                if si is not None and len(si.on_wait) > 1:
                    waits = list(si.on_wait)
                    for w in waits[:-1]:
                        count += 1
                        nop = bass_rust.InstNoOp(
                            name=f"wsplit-{count}", ins=[], outs=[])
                        nop.engine = inst.engine
                        nop.sync_info = bass_rust.SyncInfo(
                            on_wait=[w], on_update=[])
                        nop.bass_nofuse = True
                        new.append(nop)
                    si.on_wait = [waits[-1]]
                new.append(inst)
            blk.instructions = new
    return count


# -------------------------------------------------------------- host side
def _prep_inputs(x, qkv_w, o_w):
    """Build per-core input maps (numpy, fp32)."""
    q_sz = NH * HD  # 2048
    kv_sz = QG * HD  # 512
    inv_freq = (1.0 / (THETA ** (np.arange(0, HD, 2, dtype=np.float32) / HD))).astype(
        np.float32)
    pos = np.arange(T, dtype=np.float32)
    ang = pos[:, None] * inv_freq[None, :]  # [T, 64] fp32
    cos = np.cos(ang).astype(np.float32)    # [T, 64]
    sin = np.sin(ang).astype(np.float32)
    # transposed-layout tables [128, T]; sin sign-folded for rotate-half:
    # out[0:64] = x1*cos - x2*sin ; out[64:128] = x2*cos + x1*sin
    cosT = np.concatenate([cos.T, cos.T], axis=0)          # [128, T]
    sinT = np.concatenate([-sin.T, sin.T], axis=0)         # [128, T]
    cosT = np.ascontiguousarray(cosT)
    sinT = np.ascontiguousarray(sinT)

    xT = [np.ascontiguousarray(x[b].T) for b in range(B)]  # [D, T] each

    in_maps = []
    for c in range(N_CORES):
        b = c % B
        tp = c // B
        hq0 = tp * LOCAL_H * HD
        g0 = tp * LOCAL_G * HD
        wq_rows = qkv_w[hq0:hq0 + LOCAL_H * HD]                      # [1024, D]
        wk_rows = qkv_w[q_sz + g0:q_sz + g0 + LOCAL_G * HD]          # [256, D]
        wv_rows = qkv_w[q_sz + kv_sz + g0:q_sz + kv_sz + g0 + LOCAL_G * HD]
        wo_cols = o_w[:, tp * LOCAL_H * HD:(tp + 1) * LOCAL_H * HD]  # [D, 1024]
        in_maps.append({
            "xT": xT[b],
            "wq": np.ascontiguousarray(wq_rows.T),   # [D, 1024]
            "wk": np.ascontiguousarray(wk_rows.T),   # [D, 256]
            "wv": np.ascontiguousarray(wv_rows.T),   # [D, 256]
            "wo": np.ascontiguousarray(wo_cols.T),   # [1024, D]
            "cosT": cosT,
            "sinT": sinT,
        })
    return in_maps


_RUNNER_CACHE = {}


def _get_runner(iters=1):
    if iters in _RUNNER_CACHE:
        return _RUNNER_CACHE[iters]
    import jax
    import concourse.mybir as mybir
    from jax.sharding import Mesh, PartitionSpec, NamedSharding
    from jax.experimental.shard_map import shard_map
    from concourse.bass2jax import (
        _bass_exec_p, install_neuronx_cc_hook, partition_id_tensor)

    nc = _build_nc(iters)
    install_neuronx_cc_hook()
    partition_name = (
        nc.partition_id_tensor.name if nc.partition_id_tensor else None)
    in_names, out_names, out_avals = [], [], []
    for alloc in nc.m.functions[0].allocations:
        if not isinstance(alloc, mybir.MemoryLocationSet):
            continue
        name = alloc.memorylocations[0].name
        if alloc.kind == "ExternalInput":
            if name != partition_name:
                in_names.append(name)
        elif alloc.kind == "ExternalOutput":
            out_names.append(name)
            out_avals.append(jax.core.ShapedArray(
                tuple(alloc.tensor_shape), mybir.dt.np(alloc.dtype)))
    n_params, n_outs = len(in_names), len(out_names)
    all_in = in_names + out_names + ([partition_name] if partition_name else [])

    def _body(*args):
        operands = list(args)
        if partition_name is not None:
            operands.append(partition_id_tensor())
        outs = _bass_exec_p.bind(
            *operands, out_avals=tuple(out_avals), in_names=tuple(all_in),
            out_names=tuple(out_names), lowering_input_output_aliases=(),
            sim_require_finite=True, sim_require_nnan=True, nc=nc)
        return tuple(outs)

    devices = jax.devices()[:N_CORES]
    mesh = Mesh(np.asarray(devices), ("core",))
    sharded = jax.jit(
        shard_map(
            _body, mesh=mesh,
            in_specs=(PartitionSpec("core"),) * (n_params + n_outs),
            out_specs=(PartitionSpec("core"),) * n_outs, check_rep=False),
        keep_unused=True)
    sh = NamedSharding(mesh, PartitionSpec("core"))
    runner = {
        "sharded": sharded, "sh": sh, "in_names": in_names,
        "out_names": out_names, "out_avals": out_avals, "jax": jax,
        "devices": devices,
    }
    _RUNNER_CACHE[iters] = runner
    return runner


def _put_sharded(r, per_core_arrays):
    """Device-put per-core shards directly (no on-device slicing)."""
    jax = r["jax"]
    devices = r["devices"]
    shards = [jax.device_put(a, d) for a, d in zip(per_core_arrays, devices)]
    shp = per_core_arrays[0].shape
    global_shape = (N_CORES * shp[0],) + tuple(shp[1:])
    return jax.make_array_from_single_device_arrays(global_shape, r["sh"], shards)


def run_sharded(in_maps, iters=1):
    """Execute the SPMD kernel; returns list of per-core output dicts."""
    r = _get_runner(iters)
    jax = r["jax"]
    dev = [_put_sharded(r, [np.asarray(m[nm]) for m in in_maps])
           for nm in r["in_names"]]
    dev += [_put_sharded(r, [np.zeros(av.shape, av.dtype)] * N_CORES)
            for av in r["out_avals"]]
    outs = r["sharded"](*dev)
    jax.block_until_ready(outs)
    res = []
    for c in range(N_CORES):
        res.append({
            nm: np.asarray(outs[i]).reshape(N_CORES, *r["out_avals"][i].shape)[c]
            for i, nm in enumerate(r["out_names"])})
    return res


def kernel(x, padding_mask, qkv_w, o_w):
    """Full-input entry point: shards across 8 NeuronCores internally."""
    x = np.asarray(x, dtype=np.float32)
    qkv_w = np.asarray(qkv_w, dtype=np.float32)
    o_w = np.asarray(o_w, dtype=np.float32)
    # padding_mask is all-ones for this problem spec; attention ignores it.
    in_maps = _prep_inputs(x, qkv_w, o_w)
    res = run_sharded(in_maps, iters=1)
    out = np.empty((B, T, D), dtype=np.float32)
    for b in range(B):
        out[b] = (res[b]["yT"] + res[b + B]["yT"]).T
    return out
